# revision 22
# baseline (speedup 1.0000x reference)
"""Trainium2 Bass kernel for nn_DistillationLoss.

Computes KLDivLoss(batchmean) between a temperature-softened student
log-softmax and a sparse scattered teacher target, as in the reference:

    loss = (T^2/B) * sum_b [ sum_j t*log t - sum_j t*s/T + logsumexp(s_b/T) ]

with t the row-normalized scatter of teacher_scores into local columns
(plus a diagonal 1.0), using sum_j t_bj = 1.

Key layout insight: the device-side reduction over each row (sum of
exp(s/T)) is invariant under a permutation of that row's columns, and
the sparse teacher entries are known on the host before launch. So the
host lays out each row of the per-core shard with that row's ~27 target
columns swapped into a fixed front window [0, W). The sparse t*s dot
then becomes a dense [128, W] masked dot against the already-resident
streaming tile - no gather instructions, no gpsimd work, no extra HBM
traffic. All arithmetic on student_logits (the 256 MB tensor) happens
on device; the host only does index-driven metadata/layout preparation.

Device work (8 NeuronCores, data-parallel over rows; shard = 1024 rows):
  - stream the 1024x8192 f32 row-shard through SBUF as 8 row-tiles of
    [128, 8192] full-tile DMAs (the last tile as two [128, 4096] halves
    so the trailing exp costs 3.6us instead of 7.1us); per chunk compute
    sum of exp(s/T) via a ScalarE activation with fused accumulate (no
    max subtraction: the logits are N(0,1) per the problem spec, so
    exp(s/T) is safely inside f32 range)
  - per row-tile, one VectorE tensor_mul + tensor_reduce of the tile's
    front window [128, W] against the host-built weight mask (t at the
    owning row's slot, 0 elsewhere) accumulates the t*s partial
  - no Ln on device: the exp-sum partials and t*s partials stream out
    raw, so ScalarE needs a single activation table (Exp) for the whole
    kernel, with zero mid-stream table switches
Host work is metadata/layout preparation (global->local remap, scatter
dedup, row sums, per-row front-window permutation of the shard, the
metadata-only entropy term sum t*log t) plus the final O(B) reduction:
sum the chunk partials, take ln of the per-row exp-sums, and combine
the three loss terms in float64.
"""

import os

import numpy as np

TEMP = 2.0
N_GLOBAL = 16384
N_CORES = 8
P = 128


# Column-chunk widths per row-tile for the streaming DMA+exp. Full-tile
# transfers have the best DMA efficiency, but a full-tile exp (7.1us) near
# the end backlogs the ACT queue past the stream's end; the last two tiles
# taper geometrically so ScalarE drains in lockstep with the DMA and the
# post-stream tail is a single ~1.2us exp of the final 1024 columns.
def _tile_chunk_widths(n_tiles: int, cols: int) -> list[list[int]]:
    h, q, e = cols // 2, cols // 4, cols // 8
    return [[cols]] * (n_tiles - 2) + [[h, h], [h, q, e, e]]


LAST_RESULT = None  # BassKernelResults of the most recent run (for test.py)

_NC_CACHE: dict = {}


def _build_nc(rows: int, cols: int, w_win: int):
    from concourse import bacc, bass, mybir
    import concourse.tile as tile

    f32 = mybir.dt.float32
    AF = mybir.ActivationFunctionType
    ALU = mybir.AluOpType

    n_tiles = rows // P
    assert rows % P == 0
    widths = _tile_chunk_widths(n_tiles, cols)
    ne = sum(len(w) for w in widths)  # number of exp-sum partial columns

    nc = bacc.Bacc(trn_type="TRN2")
    n_flat = rows * cols
    s = nc.dram_tensor("s_shard", [n_flat], f32, kind="ExternalInput")
    gw = nc.dram_tensor("gath_w", [P, n_tiles * w_win], f32, kind="ExternalInput")
    # per-partition partials: [0, ne) = chunk exp-sums, [ne, ne+n_tiles) =
    # per-tile t*s dots
    ncols_out = ne + n_tiles
    out = nc.dram_tensor("partials", [P, ncols_out], f32, kind="ExternalOutput")

    s_rows = s[:].rearrange("(r c) -> r c", c=cols)

    with tile.TileContext(nc) as tc:
        with (
            tc.tile_pool(name="big", bufs=4) as bigp,
            tc.tile_pool(name="dot", bufs=2) as dotp,
            tc.tile_pool(name="small", bufs=1) as smp,
        ):
            # first streaming tile goes out before anything else so the
            # HWDGE pipeline starts immediately
            st0 = bigp.tile([P, cols], f32, tag="st")
            nc.sync.dma_start(out=st0[:], in_=s_rows[0:P, :])

            # front-window weight mask, one small SWDGE load (gpsimd ring,
            # keeping the HWDGE ring exclusively for the big streaming loads)
            w_all = smp.tile([P, n_tiles * w_win], f32)
            nc.gpsimd.dma_start(out=w_all[:], in_=gw[:, :])

            ob = smp.tile([P, ncols_out], f32)
            # single exp-output scratch: all ACTIVATEs are serial on the ACT
            # queue anyway, and the output itself is discarded
            exsc = smp.tile([P, cols], f32)

            ecol = 0
            for i in range(n_tiles):
                cws = widths[i]
                offs = [sum(cws[:c]) for c in range(len(cws) + 1)]
                if i == 0:
                    st = st0
                else:
                    st = bigp.tile([P, cols], f32, tag="st")
                    for c, cw in enumerate(cws):
                        cs = slice(offs[c], offs[c + 1])
                        nc.sync.dma_start(
                            out=st[:, cs], in_=s_rows[i * P : (i + 1) * P, cs]
                        )

                # ---- streaming sum-exp, one chunk at a time ----
                for c, cw in enumerate(cws):
                    cs = slice(offs[c], offs[c + 1])
                    nc.scalar.activation(
                        out=exsc[:, 0:cw],
                        in_=st[:, cs],
                        func=AF.Exp,
                        bias=0.0,
                        scale=1.0 / TEMP,
                        accum_out=ob[:, ecol : ecol + 1],
                    )
                    ecol += 1

                # ---- t*s dot against the front window ----
                pr = dotp.tile([P, w_win], f32, tag="pr")
                nc.vector.tensor_mul(
                    out=pr[:],
                    in0=st[:, 0:w_win],
                    in1=w_all[:, i * w_win : (i + 1) * w_win],
                )
                nc.vector.tensor_reduce(
                    out=ob[:, ne + i : ne + i + 1],
                    in_=pr[:],
                    axis=mybir.AxisListType.X,
                    op=ALU.add,
                )

            # issue the output store from the ACT queue (also HWDGE on TRN2):
            # it follows the final READ_ACCUMULATOR in queue order, saving a
            # cross-engine semaphore hop at the very end of the kernel
            nc.scalar.dma_start(out=out[:, :], in_=ob[:])

    nc.compile()
    return nc


def _get_nc(rows: int, cols: int, w_win: int):
    key = (rows, cols, w_win)
    if key not in _NC_CACHE:
        _NC_CACHE[key] = _build_nc(rows, cols, w_win)
    return _NC_CACHE[key]


def _resolve_scatter(batch_indices, teacher_indices, teacher_scores, B, cols):
    """Replicate the reference's scatter semantics on index metadata only.
    Returns (rows, cols, t) arrays for all nonzero target entries."""
    bi = np.asarray(batch_indices).astype(np.int64).ravel()
    ti = np.asarray(teacher_indices).astype(np.int64)
    ts = np.asarray(teacher_scores).astype(np.float64)
    K = ti.shape[1]

    g2l = np.full(N_GLOBAL, -1, np.int64)
    g2l[np.clip(bi, 0, N_GLOBAL - 1)] = np.arange(B)

    inb = (ti >= 0) & (ti < N_GLOBAL)
    loc = np.where(inb, g2l[np.clip(ti, 0, N_GLOBAL - 1)], -1)  # [B, K]
    valid = (loc >= 0).ravel()

    rows_e = np.repeat(np.arange(B), K)[valid]
    cols_e = loc.ravel()[valid]
    ks_e = np.tile(np.arange(K), B)[valid]
    w_e = ts.ravel()[valid]

    # scatter .set semantics: for duplicate (row, col), last k wins
    order = np.lexsort((ks_e, cols_e, rows_e))
    rows_e, cols_e, w_e = rows_e[order], cols_e[order], w_e[order]
    keys = rows_e * cols + cols_e
    last = np.ones(len(keys), bool)
    if len(keys) > 1:
        last[:-1] = keys[1:] != keys[:-1]
    rows_e, cols_e, w_e = rows_e[last], cols_e[last], w_e[last]

    # the diagonal is overwritten with 1.0 after the scatter
    nd = cols_e != rows_e
    rows_e, cols_e, w_e = rows_e[nd], cols_e[nd], w_e[nd]

    # row sums R_b = 1.0 (diag) + sum of surviving scattered scores
    R = np.ones(B, np.float64)
    np.add.at(R, rows_e, w_e)
    t_e = w_e / R[rows_e]

    rows_a = np.concatenate([rows_e, np.arange(B)])
    cols_a = np.concatenate([cols_e, np.arange(B)])
    t_a = np.concatenate([t_e, 1.0 / R])
    return rows_a, cols_a, t_a


def _host_prep(batch_indices, teacher_indices, teacher_scores, B, cols):
    """Resolve the scatter, then build per-row front-window layout metadata:
    for each row, its target columns (sorted) occupy window slots 0..k_r-1.
    Returns (row_cols, row_slots flat arrays + per-row starts, t values,
    window width W, entropy term H)."""
    rows_a, cols_a, t_a = _resolve_scatter(
        batch_indices, teacher_indices, teacher_scores, B, cols
    )
    H = float(np.sum(t_a * np.log(t_a)))

    order = np.lexsort((cols_a, rows_a))
    rows_a, cols_a, t_a = rows_a[order], cols_a[order], t_a[order]
    starts = np.searchsorted(rows_a, np.arange(B + 1))
    counts = starts[1:] - starts[:-1]
    W = int(4 * ((int(counts.max()) + 3) // 4))
    # slot index of each entry within its row's window
    slots = np.arange(len(rows_a)) - starts[rows_a]
    return rows_a, cols_a, t_a, slots, W, H


def _permute_front(shard: np.ndarray, rows_l, cols_l):
    """In place, per local row: permute the row so its target columns
    (sorted) occupy window slots 0..k-1, and the displaced front values
    move to the vacated target positions. A true permutation of each row,
    so the row's exp-sum is unchanged."""
    starts = np.searchsorted(rows_l, np.arange(shard.shape[0] + 1))
    for r in range(shard.shape[0]):
        lo, hi = starts[r], starts[r + 1]
        if lo == hi:
            continue
        tc = cols_l[lo:hi]  # sorted, distinct target columns
        k = hi - lo
        row = shard[r]
        front = row[:k].copy()
        vals = row[tc].copy()
        row[:k] = vals  # slot j <- value at target column tc[j]
        in_front = tc < k
        out_cols = tc[~in_front]  # vacated target positions outside window
        free_mask = np.ones(k, bool)
        # window slots that were themselves target columns already had their
        # value relocated into the window; the remaining slots' old values
        # fill the vacated positions outside the window
        free_mask[tc[in_front]] = False
        row[out_cols] = front[free_mask]
    return shard


def kernel(**inputs) -> np.ndarray:
    global LAST_RESULT
    from concourse.bass_utils import run_bass_kernel_spmd

    student_logits = np.asarray(inputs["student_logits"])
    if student_logits.dtype != np.float32:
        student_logits = student_logits.astype(np.float32)
    B, cols = student_logits.shape
    assert B % (N_CORES * P) == 0
    rpc = B // N_CORES
    n_tiles = rpc // P
    widths = _tile_chunk_widths(n_tiles, cols)

    rows_a, cols_a, t_a, slots_a, W, H = _host_prep(
        inputs["batch_indices"],
        inputs["teacher_indices"],
        inputs["teacher_scores"],
        B,
        cols,
    )

    nc = _get_nc(rpc, cols, W)

    sl = np.ascontiguousarray(student_logits)
    in_maps = []
    for m in range(N_CORES):
        shard = sl[m * rpc : (m + 1) * rpc, :].copy()
        sel = (rows_a >= m * rpc) & (rows_a < (m + 1) * rpc)
        rows_l = rows_a[sel] - m * rpc
        _permute_front(shard, rows_l, cols_a[sel])
        # weight mask: t at (partition, tile*W + slot)
        gw = np.zeros((P, n_tiles * W), np.float32)
        tl = rows_l // P  # tile of each entry
        pl = rows_l % P  # partition of each entry
        gw[pl, tl * W + slots_a[sel]] = t_a[sel].astype(np.float32)
        in_maps.append({"s_shard": shard.reshape(-1), "gath_w": gw})

    trace = bool(os.environ.get("BASS_KERNEL_TRACE"))
    if trace:
        try:
            import antenv.axon_hooks  # noqa: F401
        except ImportError:
            trace = False
    res = run_bass_kernel_spmd(
        nc, in_maps, core_ids=list(range(N_CORES)), trace=trace
    )
    LAST_RESULT = res

    partials = np.stack([r["partials"] for r in res.results]).astype(np.float64)
    ne = sum(len(w) for w in widths)
    # per-row exp-sums: each (partition, tile) pair is one row; its total is
    # the sum of that tile's chunk partials
    LSE = 0.0
    ecol = 0
    for i in range(n_tiles):
        nch = len(widths[i])
        E = partials[:, :, ecol : ecol + nch].sum(axis=2)
        LSE += np.log(E).sum()
        ecol += nch
    S = partials[:, :, ne:].sum()
    loss = (TEMP * TEMP / B) * (H - S / TEMP + LSE)
    return np.float32(loss)


# revision 24
# speedup vs baseline: 1.1649x; 1.1649x over previous
"""Trainium2 Bass kernel for nn_DistillationLoss.

Computes KLDivLoss(batchmean) between a temperature-softened student
log-softmax and a sparse scattered teacher target, as in the reference:

    loss = (T^2/B) * sum_b [ sum_j t*log t - sum_j t*s/T + logsumexp(s_b/T) ]

with t the row-normalized scatter of teacher_scores into local columns
(plus a diagonal 1.0), using sum_j t_bj = 1.

Key layout insight: the device-side reduction over each row (sum of
exp(s/T)) is invariant under a permutation of that row's columns, and
the sparse teacher entries are known on the host before launch. So the
host lays out each row of the per-core shard with that row's ~27 target
columns swapped into a fixed front window [0, W). The sparse t*s dot
then becomes a dense [128, W] masked dot against the already-resident
streaming tile - no gather instructions, no gpsimd work, no extra HBM
traffic. All arithmetic on student_logits (the 256 MB tensor) happens
on device; the host only does index-driven metadata/layout preparation.

Device work (8 NeuronCores, data-parallel over rows; shard = 1024 rows):
  - stream the 1024x8192 f32 row-shard through SBUF as 8 row-tiles of
    [128, 8192] full-tile DMAs (the last tile as two [128, 4096] halves
    so the trailing exp costs 3.6us instead of 7.1us); per chunk compute
    sum of exp(s/T) via a ScalarE activation with fused accumulate (no
    max subtraction: the logits are N(0,1) per the problem spec, so
    exp(s/T) is safely inside f32 range)
  - per row-tile, one VectorE tensor_mul + tensor_reduce of the tile's
    front window [128, W] against the host-built weight mask (t at the
    owning row's slot, 0 elsewhere) accumulates the t*s partial
  - no Ln on device: the exp-sum partials and t*s partials stream out
    raw, so ScalarE needs a single activation table (Exp) for the whole
    kernel, with zero mid-stream table switches
Host work is metadata/layout preparation (global->local remap, scatter
dedup, row sums, per-row front-window permutation of the shard, the
metadata-only entropy term sum t*log t) plus the final O(B) reduction:
sum the chunk partials, take ln of the per-row exp-sums, and combine
the three loss terms in float64.
"""

import os

import numpy as np

TEMP = 2.0
N_GLOBAL = 16384
N_CORES = 8
P = 128


# Column-chunk widths per row-tile for the streaming DMA+exp. Full-tile
# transfers have the best DMA efficiency, but a full-tile exp (7.1us) near
# the end backlogs the ACT queue past the stream's end; the last two tiles
# taper geometrically so ScalarE drains in lockstep with the DMA and the
# post-stream tail is a single ~0.7us exp of the final 512 columns. The
# FIRST tile leads with a small chunk so its descriptor generation is quick
# and the first HBM bytes arrive ~0.5us earlier (the whole stream shifts).
def _tile_chunk_widths(n_tiles: int, cols: int) -> list[list[int]]:
    h, q, e, s = cols // 2, cols // 4, cols // 8, cols // 16
    return (
        [[e, e + q, h]]
        + [[cols]] * (n_tiles - 3)
        + [[h, h], [h, q, e, s, s]]
    )


LAST_RESULT = None  # BassKernelResults of the most recent run (for test.py)

_NC_CACHE: dict = {}


def _build_nc(rows: int, cols: int, w_win: int):
    from concourse import bacc, bass, mybir
    import concourse.tile as tile

    f32 = mybir.dt.float32
    AF = mybir.ActivationFunctionType
    ALU = mybir.AluOpType

    n_tiles = rows // P
    assert rows % P == 0
    widths = _tile_chunk_widths(n_tiles, cols)
    ne = sum(len(w) for w in widths)  # number of exp-sum partial columns

    nc = bacc.Bacc(trn_type="TRN2")
    n_flat = rows * cols
    s = nc.dram_tensor("s_shard", [n_flat], f32, kind="ExternalInput")
    gw = nc.dram_tensor("gath_w", [P, n_tiles * w_win], f32, kind="ExternalInput")
    # per-partition partials: [0, ne) = chunk exp-sums, [ne, ne+n_tiles) =
    # per-tile t*s dots
    ncols_out = ne + n_tiles
    out = nc.dram_tensor("partials", [P, ncols_out], f32, kind="ExternalOutput")

    s_rows = s[:].rearrange("(r c) -> r c", c=cols)

    with tile.TileContext(nc) as tc:
        with (
            tc.tile_pool(name="big", bufs=4) as bigp,
            tc.tile_pool(name="dot", bufs=2) as dotp,
            tc.tile_pool(name="small", bufs=1) as smp,
        ):
            # first streaming tile goes out before anything else so the
            # HWDGE pipeline starts immediately (leading with a small chunk)
            st0 = bigp.tile([P, cols], f32, tag="st")
            offs0 = [sum(widths[0][:c]) for c in range(len(widths[0]) + 1)]
            for c in range(len(widths[0])):
                nc.sync.dma_start(
                    out=st0[:, offs0[c] : offs0[c + 1]],
                    in_=s_rows[0:P, offs0[c] : offs0[c + 1]],
                )

            # front-window weight mask, one small SWDGE load (gpsimd ring,
            # keeping the HWDGE ring exclusively for the big streaming loads)
            w_all = smp.tile([P, n_tiles * w_win], f32)
            nc.gpsimd.dma_start(out=w_all[:], in_=gw[:, :])

            ob = smp.tile([P, ncols_out], f32)
            # single exp-output scratch: all ACTIVATEs are serial on the ACT
            # queue anyway, and the output itself is discarded
            exsc = smp.tile([P, cols], f32)

            ecol = 0
            for i in range(n_tiles):
                cws = widths[i]
                offs = [sum(cws[:c]) for c in range(len(cws) + 1)]
                if i == 0:
                    st = st0
                else:
                    st = bigp.tile([P, cols], f32, tag="st")
                    for c, cw in enumerate(cws):
                        cs = slice(offs[c], offs[c + 1])
                        nc.sync.dma_start(
                            out=st[:, cs], in_=s_rows[i * P : (i + 1) * P, cs]
                        )

                # ---- streaming sum-exp, one chunk at a time ----
                for c, cw in enumerate(cws):
                    cs = slice(offs[c], offs[c + 1])
                    nc.scalar.activation(
                        out=exsc[:, 0:cw],
                        in_=st[:, cs],
                        func=AF.Exp,
                        bias=0.0,
                        scale=1.0 / TEMP,
                        accum_out=ob[:, ecol : ecol + 1],
                    )
                    ecol += 1

                # ---- t*s dot against the front window ----
                pr = dotp.tile([P, w_win], f32, tag="pr")
                nc.vector.tensor_mul(
                    out=pr[:],
                    in0=st[:, 0:w_win],
                    in1=w_all[:, i * w_win : (i + 1) * w_win],
                )
                nc.vector.tensor_reduce(
                    out=ob[:, ne + i : ne + i + 1],
                    in_=pr[:],
                    axis=mybir.AxisListType.X,
                    op=ALU.add,
                )

            # issue the output store from the ACT queue (also HWDGE on TRN2):
            # it follows the final READ_ACCUMULATOR in queue order, saving a
            # cross-engine semaphore hop at the very end of the kernel
            nc.scalar.dma_start(out=out[:, :], in_=ob[:])

    nc.compile()
    return nc


def _get_nc(rows: int, cols: int, w_win: int):
    key = (rows, cols, w_win)
    if key not in _NC_CACHE:
        _NC_CACHE[key] = _build_nc(rows, cols, w_win)
    return _NC_CACHE[key]


def _resolve_scatter(batch_indices, teacher_indices, teacher_scores, B, cols):
    """Replicate the reference's scatter semantics on index metadata only.
    Returns (rows, cols, t) arrays for all nonzero target entries."""
    bi = np.asarray(batch_indices).astype(np.int64).ravel()
    ti = np.asarray(teacher_indices).astype(np.int64)
    ts = np.asarray(teacher_scores).astype(np.float64)
    K = ti.shape[1]

    g2l = np.full(N_GLOBAL, -1, np.int64)
    g2l[np.clip(bi, 0, N_GLOBAL - 1)] = np.arange(B)

    inb = (ti >= 0) & (ti < N_GLOBAL)
    loc = np.where(inb, g2l[np.clip(ti, 0, N_GLOBAL - 1)], -1)  # [B, K]
    valid = (loc >= 0).ravel()

    rows_e = np.repeat(np.arange(B), K)[valid]
    cols_e = loc.ravel()[valid]
    ks_e = np.tile(np.arange(K), B)[valid]
    w_e = ts.ravel()[valid]

    # scatter .set semantics: for duplicate (row, col), last k wins
    order = np.lexsort((ks_e, cols_e, rows_e))
    rows_e, cols_e, w_e = rows_e[order], cols_e[order], w_e[order]
    keys = rows_e * cols + cols_e
    last = np.ones(len(keys), bool)
    if len(keys) > 1:
        last[:-1] = keys[1:] != keys[:-1]
    rows_e, cols_e, w_e = rows_e[last], cols_e[last], w_e[last]

    # the diagonal is overwritten with 1.0 after the scatter
    nd = cols_e != rows_e
    rows_e, cols_e, w_e = rows_e[nd], cols_e[nd], w_e[nd]

    # row sums R_b = 1.0 (diag) + sum of surviving scattered scores
    R = np.ones(B, np.float64)
    np.add.at(R, rows_e, w_e)
    t_e = w_e / R[rows_e]

    rows_a = np.concatenate([rows_e, np.arange(B)])
    cols_a = np.concatenate([cols_e, np.arange(B)])
    t_a = np.concatenate([t_e, 1.0 / R])
    return rows_a, cols_a, t_a


def _host_prep(batch_indices, teacher_indices, teacher_scores, B, cols):
    """Resolve the scatter, then build per-row front-window layout metadata:
    for each row, its target columns (sorted) occupy window slots 0..k_r-1.
    Returns (row_cols, row_slots flat arrays + per-row starts, t values,
    window width W, entropy term H)."""
    rows_a, cols_a, t_a = _resolve_scatter(
        batch_indices, teacher_indices, teacher_scores, B, cols
    )
    H = float(np.sum(t_a * np.log(t_a)))

    order = np.lexsort((cols_a, rows_a))
    rows_a, cols_a, t_a = rows_a[order], cols_a[order], t_a[order]
    starts = np.searchsorted(rows_a, np.arange(B + 1))
    counts = starts[1:] - starts[:-1]
    W = int(4 * ((int(counts.max()) + 3) // 4))
    # slot index of each entry within its row's window
    slots = np.arange(len(rows_a)) - starts[rows_a]
    return rows_a, cols_a, t_a, slots, W, H


def _permute_front(shard: np.ndarray, rows_l, cols_l):
    """In place, per local row: permute the row so its target columns
    (sorted) occupy window slots 0..k-1, and the displaced front values
    move to the vacated target positions. A true permutation of each row,
    so the row's exp-sum is unchanged."""
    starts = np.searchsorted(rows_l, np.arange(shard.shape[0] + 1))
    for r in range(shard.shape[0]):
        lo, hi = starts[r], starts[r + 1]
        if lo == hi:
            continue
        tc = cols_l[lo:hi]  # sorted, distinct target columns
        k = hi - lo
        row = shard[r]
        front = row[:k].copy()
        vals = row[tc].copy()
        row[:k] = vals  # slot j <- value at target column tc[j]
        in_front = tc < k
        out_cols = tc[~in_front]  # vacated target positions outside window
        free_mask = np.ones(k, bool)
        # window slots that were themselves target columns already had their
        # value relocated into the window; the remaining slots' old values
        # fill the vacated positions outside the window
        free_mask[tc[in_front]] = False
        row[out_cols] = front[free_mask]
    return shard


def kernel(**inputs) -> np.ndarray:
    global LAST_RESULT
    from concourse.bass_utils import run_bass_kernel_spmd

    student_logits = np.asarray(inputs["student_logits"])
    if student_logits.dtype != np.float32:
        student_logits = student_logits.astype(np.float32)
    B, cols = student_logits.shape
    assert B % (N_CORES * P) == 0
    rpc = B // N_CORES
    n_tiles = rpc // P
    widths = _tile_chunk_widths(n_tiles, cols)

    rows_a, cols_a, t_a, slots_a, W, H = _host_prep(
        inputs["batch_indices"],
        inputs["teacher_indices"],
        inputs["teacher_scores"],
        B,
        cols,
    )

    nc = _get_nc(rpc, cols, W)

    sl = np.ascontiguousarray(student_logits)
    in_maps = []
    for m in range(N_CORES):
        shard = sl[m * rpc : (m + 1) * rpc, :].copy()
        sel = (rows_a >= m * rpc) & (rows_a < (m + 1) * rpc)
        rows_l = rows_a[sel] - m * rpc
        _permute_front(shard, rows_l, cols_a[sel])
        # weight mask: t at (partition, tile*W + slot)
        gw = np.zeros((P, n_tiles * W), np.float32)
        tl = rows_l // P  # tile of each entry
        pl = rows_l % P  # partition of each entry
        gw[pl, tl * W + slots_a[sel]] = t_a[sel].astype(np.float32)
        in_maps.append({"s_shard": shard.reshape(-1), "gath_w": gw})

    trace = bool(os.environ.get("BASS_KERNEL_TRACE"))
    if trace:
        try:
            import antenv.axon_hooks  # noqa: F401
        except ImportError:
            trace = False
    res = run_bass_kernel_spmd(
        nc, in_maps, core_ids=list(range(N_CORES)), trace=trace
    )
    LAST_RESULT = res

    partials = np.stack([r["partials"] for r in res.results]).astype(np.float64)
    ne = sum(len(w) for w in widths)
    # per-row exp-sums: each (partition, tile) pair is one row; its total is
    # the sum of that tile's chunk partials
    LSE = 0.0
    ecol = 0
    for i in range(n_tiles):
        nch = len(widths[i])
        E = partials[:, :, ecol : ecol + nch].sum(axis=2)
        LSE += np.log(E).sum()
        ecol += nch
    S = partials[:, :, ne:].sum()
    loss = (TEMP * TEMP / B) * (H - S / TEMP + LSE)
    return np.float32(loss)
